# revision 19
# baseline (speedup 1.0000x reference)
"""CapsuleLayer (dynamic routing) Trainium2 kernel.

Strategy: data-parallel over batch across 8 NeuronCores (32 samples/core).
Per core:
  - u_hat via PE matmuls: 16 input-capsules packed per 128-wide contraction.
    Block-diagonal stationary x tiles are built ON-CHIP by GPSIMD
    (mask * broadcast of dense x), fp16; W replicated.
  - s of routing iter 1 (uniform coupling) comes straight from dense-x
    matmuls accumulated over all 1152 capsules in PSUM.
  - Routing iters 2,3 fused on-chip: u_hat resident in SBUF (fp16, e-major
    free layout); agreement b = sum_e u*v via elementwise mul + pairwise
    tree-reduce; softmax on DVE/ACT; coupling sum_i c*u via selector
    matmuls accumulating all 32 samples into one PSUM tile; squash batched
    over 32 samples.

Self-contained: needs numpy + the concourse package on sys.path.
"""
import numpy as np
import concourse.bacc as bacc
import concourse.tile as tile
from concourse import mybir
from concourse.bass_utils import run_bass_kernel_spmd

B, Nin, Din, Nout, Dout = 256, 1152, 8, 10, 16
NCORES = 8
BL = B // NCORES      # 32 samples per core
G = Nin // 16         # 72 groups of 16 input capsules
GC = G // 3           # 24 chunks of 3 groups
JE = Nout * Dout      # 160 free elems, e-major: fe = e*10 + j
NBG = BL // 8         # 4 sample-groups of 8
EPS = 1e-7

F16 = mybir.dt.float16
F32 = mybir.dt.float32
AL = mybir.AluOpType
AF = mybir.ActivationFunctionType
AX = mybir.AxisListType

# engine tuning knobs
CFG = {
    "xbd": "vector",     # block-diag construction engine
    "xbd_chunk": 12,     # groups per xbd-build op
    "amul": "vector",    # (a) u*v broadcast mul
    "l1": "vector",      # first tree level
    "l23": "vector",     # tree levels 2-3
    "cu": "vector",      # (c) u*c broadcast mul
    "evac_act": 2,       # of every 3 psum evacs, how many go to ACT
}


# ----------------------------------------------------------------- host prep
def _host_prep_x(xc):
    """xc [BL, Nin, Din] fp32 -> dense transposed x [128, G, 32] f16:
    xd[il*8+d, g, b] = xc[b, g*16+il, d]"""
    t = xc.reshape(BL, G, 16, 8).transpose(2, 3, 1, 0)  # [il,d,g,b]
    return np.ascontiguousarray(t.reshape(128, G, BL)).astype(np.float16)


def _host_prep_w(W):
    """W [Nin, Nout, Din, Dout] -> wg [128, G, JE] f16:
    wg[il*8+d, g, e*10+j] = W[g*16+il, j, d, e]"""
    W6 = W.reshape(G, 16, Nout, Din, Dout).transpose(1, 3, 0, 4, 2)
    return np.ascontiguousarray(W6.reshape(128, G, JE)).astype(np.float16)


def _consts():
    il = np.arange(16)
    bl = np.arange(8)
    mask = np.zeros((16, 8, 16, 8), np.float16)   # [(il,d),(il2,bl)]
    mask[il, :, il, :] = 1.0
    mask = mask.reshape(128, 128)
    bsel4 = np.zeros((NBG, 128, 32), np.float16)  # [bg][(il,bl), b']
    vsel4 = np.zeros((NBG, 32, 128), np.float16)  # [bg][b, (il,bl)]
    for bg in range(NBG):
        for i in il:
            bsel4[bg, i * 8 + bl, bg * 8 + bl] = 1.0
            vsel4[bg, bg * 8 + bl, i * 8 + bl] = 1.0
    return mask, bsel4, vsel4


# -------------------------------------------------------------- device build
def _eng(nc, which):
    return {"vector": nc.vector, "gpsimd": nc.gpsimd, "scalar": nc.scalar}[which]


def _squash32(nc, pools, s_sb, vout_dram, final):
    """s_sb: SBUF [32, JE] fp32 (e-major), already scaled.
    Returns [32, JE] f16 v tile; if final, DMAs fp32 result instead."""
    spool = pools["small"]
    sq = spool.tile([32, JE], F32, tag="sq")
    nc.scalar.square(sq[:], s_sb[:])
    s2 = spool.tile([32, Nout], F32, tag="s2")
    nc.vector.tensor_reduce(
        s2[:], sq[:].rearrange("p (e j) -> p j e", e=Dout, j=Nout),
        axis=AX.X, op=AL.add)
    s2e = spool.tile([32, Nout], F32, tag="s2e")
    nc.vector.tensor_scalar_add(s2e[:], s2[:], EPS)
    q = spool.tile([32, Nout], F32, tag="q")
    nc.scalar.activation(q[:], s2e[:], AF.Sqrt)
    t2 = spool.tile([32, Nout], F32, tag="t2")
    nc.vector.tensor_scalar_add(t2[:], s2[:], 1.0)
    den = spool.tile([32, Nout], F32, tag="den")
    nc.vector.tensor_mul(den[:], q[:], t2[:])
    rden = spool.tile([32, Nout], F32, tag="rden")
    nc.vector.reciprocal(rden[:], den[:])
    sc = spool.tile([32, Nout], F32, tag="sc")
    nc.vector.tensor_mul(sc[:], s2[:], rden[:])
    sc_b = sc[:].unsqueeze(1).broadcast_to([32, Dout, Nout])
    s_v = s_sb[:].rearrange("p (e j) -> p e j", e=Dout, j=Nout)
    if final:
        v32 = spool.tile([32, JE], F32, tag="v32")
        nc.vector.tensor_mul(
            v32[:].rearrange("p (e j) -> p e j", e=Dout, j=Nout), s_v, sc_b)
        nc.sync.dma_start(out=vout_dram[:], in_=v32[:])
        return None
    v_new = spool.tile([32, JE], F16, tag="vbg")
    nc.vector.tensor_mul(
        v_new[:].rearrange("p (e j) -> p e j", e=Dout, j=Nout), s_v, sc_b)
    return v_new


def _routing_pass(nc, pools, k, U, b1, v_cur, bsel4_sb, vsel4_sb, vout_dram):
    wpool, spool, mpool = pools["work"], pools["small"], pools["med"]
    pS, pV = pools["psS"], pools["psV"]
    GQ = G // 4        # 18 groups per quarter-chunk
    sacc = pS.tile([32, 3, JE], F32, tag="sacc")
    first = True
    for bg in range(NBG):
        # v_rep[p=(il,bl), fe] = v_cur[bg*8+bl, fe]
        pv = pV.tile([128, JE], F32, tag="pv")
        nc.tensor.matmul(pv[:], vsel4_sb[bg], v_cur[:], start=True, stop=True)
        vrep = spool.tile([128, JE], F16, tag="vrep")
        nc.vector.tensor_copy(vrep[:], pv[:])
        for gq in range(4):
            g0 = gq * GQ
            ub = U[:, bg, g0:g0 + GQ, :]            # [128, GQ, JE]
            # (a) t1 = U * vrep  (broadcast over g)
            t1t = wpool.tile([128, GQ, JE], F16, tag="t1")
            amul_eng = CFG["amul"] if CFG["amul"] != "alt" else (
                "gpsimd" if (bg + gq) % 2 == 0 else "vector")
            _eng(nc, amul_eng).tensor_mul(
                t1t[:], ub, vrep[:].unsqueeze(1).broadcast_to([128, GQ, JE]))
            # tree-reduce over e: [128, GQ, 16, 10] -> [128, GQ, 10]
            tv = t1t[:].rearrange("p g (e j) -> p g e j", e=Dout, j=Nout)
            l1t = wpool.tile([128, GQ, 8, Nout], F16, tag="l1")
            _eng(nc, CFG["l1"]).tensor_add(
                l1t[:], tv[:, :, 0:8, :], tv[:, :, 8:16, :])
            l2t = wpool.tile([128, GQ, 4, Nout], F16, tag="l2")
            _eng(nc, CFG["l23"]).tensor_add(
                l2t[:], l1t[:, :, 0:4, :], l1t[:, :, 4:8, :])
            l3t = wpool.tile([128, GQ, 2, Nout], F16, tag="l3")
            _eng(nc, CFG["l23"]).tensor_add(
                l3t[:], l2t[:, :, 0:2, :], l2t[:, :, 2:4, :])
            bslice = b1[:, bg, g0:g0 + GQ, :]
            if k == 1:
                nc.vector.tensor_add(bslice.unsqueeze(2),
                                     l3t[:, :, 0:1, :], l3t[:, :, 1:2, :])
                zsrc = bslice
            else:
                bb2 = mpool.tile([128, GQ, Nout], F32, tag="bb2")
                nc.vector.tensor_add(bb2[:].unsqueeze(2),
                                     l3t[:, :, 0:1, :], l3t[:, :, 1:2, :])
                zt = mpool.tile([128, GQ, Nout], F32, tag="zt")
                nc.vector.tensor_add(zt[:], bslice, bb2[:])
                zsrc = zt[:]
            ex = mpool.tile([128, GQ, Nout], F32, tag="ex")
            nc.scalar.activation(ex[:], zsrc, AF.Exp)
            Zt = spool.tile([128, GQ], F32, tag="Z")
            nc.vector.tensor_reduce(Zt[:], ex[:], axis=AX.X, op=AL.add)
            rz = spool.tile([128, GQ], F32, tag="rz")
            nc.vector.reciprocal(rz[:], Zt[:])
            c2 = mpool.tile([128, GQ, Nout], F16, tag="c2")
            nc.vector.tensor_mul(
                c2[:], ex[:],
                rz[:].unsqueeze(2).broadcast_to([128, GQ, Nout]))
            # (c) cu = U * c2 (broadcast over e)
            cu = wpool.tile([128, GQ, JE], F16, tag="t1")
            cu_eng = CFG["cu"] if CFG["cu"] != "alt" else (
                "gpsimd" if (bg + gq) % 2 == 1 else "vector")
            _eng(nc, cu_eng).tensor_mul(
                cu[:].rearrange("p g (e j) -> p g e j", e=Dout, j=Nout),
                ub.rearrange("p g (e j) -> p g e j", e=Dout, j=Nout),
                c2[:].unsqueeze(2).broadcast_to([128, GQ, Dout, Nout]))
            for gc in range(GQ // 3):
                nc.tensor.matmul(
                    sacc[:], bsel4_sb[bg],
                    cu[:, 3 * gc:3 * gc + 3, :],
                    start=first, stop=(bg == NBG - 1 and gq == 3
                                       and gc == GQ // 3 - 1))
                first = False
    # collapse the 3 side-by-side partial sums
    s3 = mpool.tile([32, 3, JE], F32, tag="s3")
    nc.scalar.copy(s3[:], sacc[:])
    sa = spool.tile([32, JE], F32, tag="sa")
    nc.vector.tensor_add(sa[:], s3[:, 0, :], s3[:, 1, :])
    sb = spool.tile([32, JE], F32, tag="sb")
    nc.vector.tensor_add(sb[:], sa[:], s3[:, 2, :])
    return _squash32(nc, pools, sb, vout_dram, final=(k == 2))


def _build_program(reps=1, stages=3):
    nc = bacc.Bacc("TRN2", target_bir_lowering=False, debug=False,
                   num_devices=NCORES)
    xd = nc.dram_tensor("xd", [128, G, BL], F16, kind="ExternalInput").ap()
    wg = nc.dram_tensor("wg", [128, G, JE], F16, kind="ExternalInput").ap()
    mask = nc.dram_tensor("mask", [128, 128], F16, kind="ExternalInput").ap()
    bsel4 = nc.dram_tensor("bsel4", [NBG, 128, 32], F16,
                           kind="ExternalInput").ap()
    vsel4 = nc.dram_tensor("vsel4", [NBG, 32, 128], F16,
                           kind="ExternalInput").ap()
    vout = nc.dram_tensor("vout", [BL, JE], F32,  # e-major
                          kind="ExternalOutput").ap()

    with tile.TileContext(nc) as tc:
        with (
            tc.tile_pool(name="const", bufs=1) as cpool,
            tc.tile_pool(name="xin", bufs=2) as xpool,
            tc.tile_pool(name="u", bufs=1) as upool,
            tc.tile_pool(name="work", bufs=4) as wpool,
            tc.tile_pool(name="small", bufs=3) as spool,
            tc.tile_pool(name="med", bufs=3) as mpool,
            tc.tile_pool(name="psA", bufs=4, space="PSUM") as pA,
            tc.tile_pool(name="psS", bufs=2, space="PSUM") as pS,
            tc.tile_pool(name="psV", bufs=1, space="PSUM") as pV,
        ):
            pools = {"work": wpool, "small": spool, "med": mpool,
                     "psS": pS, "psV": pV}
            wg_sb = cpool.tile([128, G, JE], F16)
            nc.sync.dma_start(out=wg_sb[:], in_=wg[:])
            xd_sb = cpool.tile([128, G, BL], F16)
            nc.sync.dma_start(out=xd_sb[:], in_=xd[:])
            mask_sb = cpool.tile([128, 128], F16)
            nc.sync.dma_start(out=mask_sb[:], in_=mask[:])
            bsel4_sb, vsel4_sb = [], []
            for bg in range(NBG):
                bt = cpool.tile([128, 32], F16, tag=f"bsel{bg}")
                nc.sync.dma_start(out=bt[:], in_=bsel4[bg])
                bsel4_sb.append(bt[:])
                vt = cpool.tile([32, 128], F16, tag=f"vsel{bg}")
                nc.sync.dma_start(out=vt[:], in_=vsel4[bg])
                vsel4_sb.append(vt[:])

            U = upool.tile([128, NBG, G, JE], F16)
            b1 = upool.tile([128, NBG, G, Nout], F32)
            mask_v = mask_sb[:].rearrange("p (i b) -> p i b", i=16, b=8)

            def body():
                evac_rr = 0
                XC = CFG["xbd_chunk"]
                # s of iteration 1 straight from dense x (uniform coupling);
                # emitted FIRST so v1 is ready early and routing overlaps
                # with phase 0.
                s1_ps = pS.tile([32, JE], F32, tag="sacc")
                for g in range(G):
                    nc.tensor.matmul(
                        s1_ps[:],
                        xd_sb[:, g, :],
                        wg_sb[:, g, :],
                        start=(g == 0), stop=(g == G - 1))
                s1_sb = spool.tile([32, JE], F32, tag="s1sb")
                nc.scalar.mul(s1_sb[:], s1_ps[:], 0.1)
                v_cur = _squash32(nc, pools, s1_sb, vout, final=False)
                # phase 0: u_hat tiles
                for bg in range(NBG):
                    for xc0 in range(0, G, XC):
                        xbt = xpool.tile([128, XC, 16, 8], F16, tag="xbd")
                        _eng(nc, CFG["xbd"]).tensor_mul(
                            xbt[:],
                            mask_v.unsqueeze(1).broadcast_to(
                                [128, XC, 16, 8]),
                            xd_sb[:, xc0:xc0 + XC,
                                  bg * 8:(bg + 1) * 8].unsqueeze(2)
                            .broadcast_to([128, XC, 16, 8]))
                        for gc in range(XC // 3):
                            pu = pA.tile([128, 3, JE], F32, tag="pu")
                            for j3 in range(3):
                                g3 = gc * 3 + j3
                                nc.tensor.matmul(
                                    pu[:, j3, :],
                                    xbt[:, g3, :, :],
                                    wg_sb[:, xc0 + g3, :],
                                    start=True, stop=True)
                            dst = U[:, bg, xc0 + 3 * gc:xc0 + 3 * gc + 3, :]
                            evac_rr = (evac_rr + 1) % 3
                            if evac_rr < CFG["evac_act"]:
                                nc.scalar.copy(dst, pu[:])
                            else:
                                nc.vector.tensor_copy(dst, pu[:])
                if stages < 1:
                    return
                for k in (1, 2):
                    if stages < k + 1:
                        return
                    v_cur = _routing_pass(nc, pools, k, U, b1, v_cur,
                                          bsel4_sb, vsel4_sb, vout)
            if reps == 1:
                body()
            else:
                with tc.For_i(0, reps, 1):
                    body()
    nc.compile()
    return nc


_NC = None


def _get_nc():
    global _NC
    if _NC is None:
        _NC = _build_program()
    return _NC


# ------------------------------------------------------------------ entry
def make_in_maps(x, W):
    x = np.asarray(x, dtype=np.float32)
    W = np.asarray(W, dtype=np.float32)
    wg_host = _host_prep_w(W)
    mask, bsel4, vsel4 = _consts()
    in_maps = []
    for c in range(NCORES):
        xc = x[c * BL:(c + 1) * BL]
        in_maps.append({
            "xd": _host_prep_x(xc),
            "wg": wg_host,
            "mask": mask,
            "bsel4": bsel4,
            "vsel4": vsel4,
        })
    return in_maps


def kernel(x, W):
    nc = _get_nc()
    in_maps = make_in_maps(x, W)
    res = run_bass_kernel_spmd(nc, in_maps, core_ids=list(range(NCORES)))
    out = np.concatenate([res.results[c]["vout"] for c in range(NCORES)],
                         axis=0)
    # device layout is e-major [B, (e j)] -> [B, j, e]
    return out.reshape(B, Dout, Nout).transpose(0, 2, 1).astype(np.float32)


if __name__ == "__main__":
    rng = np.random.default_rng(0)
    x = rng.standard_normal((B, Nin, Din)).astype(np.float32)
    W = rng.standard_normal((Nin, Nout, Din, Dout)).astype(np.float32) * 0.35
    v = kernel(x, W)
    print("out", v.shape, v.dtype, float(np.abs(v).max()))
